# revision 15
# baseline (speedup 1.0000x reference)
"""Trainium2 Bass kernel for nn_AttentionBlock (linear attention + BatchNorm).

Math (per batch, c=256 channels, n=1024 pixels, 8 heads x 64 dims):
  qkv = w_qkv @ x                      [1536, n]
  q   = softmax(q, axis=d) * d^-0.5    (per head, over the 64 head-dims)
  k   = softmax(k, axis=n)             (per head-dim, over pixels)
  ctx = k @ (v/n)^T                    [d, e] per head
  out = ctx^T @ q                      [e, n] per head
  y   = BatchNorm(w_out @ out + b_out) (batch stats over (b, n) per channel)

Sharding: data-parallel over batch across 8 cores (4 batches each); BN batch
stats are combined with a tiny AllReduce (2 floats per channel). b_out is
skipped: BatchNorm's mean subtraction cancels any per-channel constant exactly.

Device layouts (per batch):
  x      [c, n]           c on partitions (2 tiles)
  q      [(h d), n]       via lhsT=w_q^T  -> exp on ACT -> expq (f32r)
  kv^T   [n, (k|v)(h d)]  via lhsT=x      -> exp_k / v copies (fp16)
  ctx_h  [d, e+1]         contraction over n (8 chunks, PSUM accum);
                          col 64 = Zk (ones column) -> per-partition norm
  out_h  [e, n]           lhsT=ctx, rhs=expq, quadrant-packed pairs of heads
  Zq     [(h d), n]       block-mask matmul; recip folds SCALE and 1/n
  final  [c, n]           lhsT=w_out^T; bn_stats per batch; AllReduce of
                          packed (mean, E[x^2]); normalize in place; DMA out.
"""

import os
import sys

import numpy as np

for _p in ("/opt/trn_rl_repo", "/root/.axon_site/_ro/trn_rl_repo"):
    if os.path.isdir(_p) and _p not in sys.path:
        sys.path.insert(0, _p)

import concourse.bacc as bacc
import concourse.tile as tile
from concourse import mybir
from concourse.bass_utils import run_bass_kernel_spmd

F32 = mybir.dt.float32
F32R = mybir.dt.float32r
FP16 = mybir.dt.float16
AF = mybir.ActivationFunctionType
ALU = mybir.AluOpType

N_CORES = 8
# B is overridable for cheap simulator runs (BASS_ATTN_B=1 -> 8 batches total).
B = int(os.environ.get("BASS_ATTN_B", "4"))  # batches per core
C = 256          # channels
NPIX = 1024      # pixels (32*32)
H = 8            # heads
D = 64           # head dim
HID = H * D      # 512
NT = NPIX // 128  # 8 n-tiles
CT = C // 128     # 2 c-tiles
QT = HID // 128   # 4 q-tiles
SCALE = D ** -0.5
BN_EPS = 1e-5
# Zq-broadcast matmul uses this instead of 1.0 so reciprocal(Zqb) directly
# yields SCALE / (n * Zq), folding the softmax scale and the v/n factor.
MASKVAL = NPIX / SCALE


def _emit(tc, x, wqkv, wout, gammab, betab, y):
    nc = tc.nc
    from contextlib import ExitStack
    ctx_stack = ExitStack()
    with ctx_stack:
        const = ctx_stack.enter_context(tc.tile_pool(name="const", bufs=1))
        xin = ctx_stack.enter_context(tc.tile_pool(name="xin", bufs=4))
        kvsb = ctx_stack.enter_context(tc.tile_pool(name="kvsb", bufs=3))
        qpool = ctx_stack.enter_context(tc.tile_pool(name="qpool", bufs=6))
        rpool = ctx_stack.enter_context(tc.tile_pool(name="rpool", bufs=5))
        cpool = ctx_stack.enter_context(tc.tile_pool(name="cpool", bufs=8))
        opool = ctx_stack.enter_context(tc.tile_pool(name="opool", bufs=6))
        fpool = ctx_stack.enter_context(tc.tile_pool(name="fpool", bufs=2 * B))
        small = ctx_stack.enter_context(tc.tile_pool(name="small", bufs=8))
        stats_p = ctx_stack.enter_context(tc.tile_pool(name="statsp", bufs=1))
        pbig = ctx_stack.enter_context(
            tc.tile_pool(name="pbig", bufs=4, space="PSUM"))
        pctx = ctx_stack.enter_context(
            tc.tile_pool(name="pctx", bufs=4, space="PSUM"))
        dpool = ctx_stack.enter_context(
            tc.tile_pool(name="dram", bufs=1, space="DRAM"))

        # ---- constants ----
        wqkv_sb = []
        for kc in range(CT):
            w = const.tile([128, 3 * HID], F32R, name=f"wqkv{kc}")
            # kv columns first so the first batch's kv matmuls start early
            for piece in range(2):
                c0 = HID + 512 * piece
                nc.sync.dma_start(out=w[:, c0:c0 + 512],
                                  in_=wqkv[128 * kc:128 * (kc + 1), c0:c0 + 512])
            nc.sync.dma_start(out=w[:, 0:HID],
                              in_=wqkv[128 * kc:128 * (kc + 1), 0:HID])
            wqkv_sb.append(w)
        wout_sb = []
        for k4 in range(HID // 128):
            w = const.tile([128, C], F32R, name=f"wout{k4}")
            nc.sync.dma_start(out=w, in_=wout[128 * k4:128 * (k4 + 1), :])
            wout_sb.append(w)
        gamma_sb, beta_sb = [], []
        for m in range(CT):
            g = const.tile([128, 1], F32, name=f"gamma{m}")
            nc.sync.dma_start(out=g, in_=gammab[128 * m:128 * (m + 1), :])
            gamma_sb.append(g)
            bb = const.tile([128, 1], F32, name=f"beta{m}")
            nc.sync.dma_start(out=bb, in_=betab[128 * m:128 * (m + 1), :])
            beta_sb.append(bb)
        bmask = const.tile([128, 128], FP16, name="bmask")
        nc.vector.memset(bmask, 0.0)
        nc.vector.memset(bmask[0:64, 0:64], MASKVAL)
        nc.vector.memset(bmask[64:128, 64:128], MASKVAL)
        eps_sb = const.tile([128, 1], F32, name="eps")
        nc.vector.memset(eps_sb, BN_EPS)


        stats_sb = [stats_p.tile([128, 2 * B, 6], F32, name=f"stats{m}")
                    for m in range(CT)]
        final_sb = [[None] * CT for _ in range(B)]

        for b in range(B):
            xc = []
            for kc in range(CT):
                xt = xin.tile([128, NPIX], F32R, name="xc")
                for hf in range(2):
                    nc.scalar.dma_start(
                        out=xt[:, 512 * hf:512 * (hf + 1)],
                        in_=x[b, 128 * kc:128 * (kc + 1),
                              512 * hf:512 * (hf + 1)])
                xc.append(xt)

            # ---- KV projection + context accumulation over n-chunks ----
            # one PSUM bank per head-pair: only one accumulation group may be
            # open per 2KB zero region per partition
            ctxu = [pctx.tile([128, D + 1], F32, name="ctxu", tag="ctxu")
                    for _ in range(4)]
            for t in range(NT):
                halves = []
                for nch in range(2):
                    hp = pbig.tile([128, 512], F32, name="kvp", tag="big")
                    for kc in range(CT):
                        nc.tensor.matmul(
                            hp,
                            lhsT=xc[kc][:, 128 * t:128 * (t + 1)],
                            rhs=wqkv_sb[kc][:, HID + 512 * nch:
                                            HID + 512 * (nch + 1)],
                            start=(kc == 0), stop=(kc == CT - 1))
                    halves.append(hp)
                expk = kvsb.tile([128, HID], FP16, name="expk")
                nc.scalar.activation(out=expk, in_=halves[0], func=AF.Exp)
                vx = kvsb.tile([128, H, D + 1], FP16, name="vx")
                nc.vector.memset(vx[:, :, D:D + 1], 1.0)
                nc.vector.tensor_copy(
                    vx[:, :, 0:D],
                    halves[1].rearrange("p (h e) -> p h e", h=H))
                for pr in range(4):
                    for j in range(2):
                        h = 2 * pr + j
                        # skip_group_check: j=0/j=1 share the bank but write
                        # disjoint partition ranges; the sim's zero-region
                        # bookkeeping ignores partition base and would raise.
                        nc.tensor.matmul(
                            ctxu[pr][64 * j:64 * (j + 1), :],
                            lhsT=expk[:, D * h:D * (h + 1)],
                            rhs=vx[:, h, :],
                            start=(t == 0), stop=(t == NT - 1),
                            tile_position=(0, 64 * j),
                            skip_group_check=True)

            # ---- context normalization (per-partition Zk) ----
            ctx_sb = []
            for pr in range(4):
                rz = small.tile([128, 1], F32, name="rz")
                nc.vector.reciprocal(rz, ctxu[pr][:, D:D + 1])
                cs = cpool.tile([128, D], FP16, name="ctxsb")
                nc.vector.tensor_scalar_mul(cs, in0=ctxu[pr][:, 0:D], scalar1=rz)
                ctx_sb.append(cs)

            # ---- Q projection, exp, Zq block-broadcast, reciprocal ----
            expq, recipb = [], []
            for t in range(QT):
                eq = qpool.tile([128, NPIX], FP16, name="expq")
                rb = rpool.tile([128, NPIX], F32, name="recipb")
                for nch in range(2):
                    qh = pbig.tile([128, 512], F32, name="qp", tag="big")
                    for kc in range(CT):
                        nc.tensor.matmul(
                            qh,
                            lhsT=wqkv_sb[kc][:, 128 * t:128 * (t + 1)],
                            rhs=xc[kc][:, 512 * nch:512 * (nch + 1)],
                            start=(kc == 0), stop=(kc == CT - 1))
                    eqh = eq[:, 512 * nch:512 * (nch + 1)]
                    nc.scalar.activation(out=eqh, in_=qh, func=AF.Exp)
                    # Zqb overwrites qh (WAR-ordered after the exp read):
                    # one 1-bank slot per half, freed right after the recip.
                    nc.tensor.matmul(qh, lhsT=bmask, rhs=eqh,
                                     start=True, stop=True)
                    nc.vector.reciprocal(
                        rb[:, 512 * nch:512 * (nch + 1)], qh)
                expq.append(eq)
                recipb.append(rb)

            # ---- out = ctx^T @ expq (quadrant-packed pairs), normalize ----
            out_sb = []
            for t in range(QT):
                os_ = opool.tile([128, NPIX], F32R, name="outsb")
                for nch in range(2):
                    oh = pbig.tile([128, 512], F32, name="ou", tag="big")
                    for j in range(2):
                        # j=0/j=1 share the bank but write disjoint partition
                        # ranges (see ctxu note on skip_group_check).
                        nc.tensor.matmul(
                            oh[64 * j:64 * (j + 1), :],
                            lhsT=ctx_sb[t][64 * j:64 * (j + 1), :],
                            rhs=expq[t][64 * j:64 * (j + 1),
                                        512 * nch:512 * (nch + 1)],
                            start=True, stop=True,
                            tile_position=(64 * j, 64 * j),
                            skip_group_check=True)
                    nc.vector.tensor_mul(
                        os_[:, 512 * nch:512 * (nch + 1)], oh,
                        recipb[t][:, 512 * nch:512 * (nch + 1)])
                out_sb.append(os_)

            # ---- final projection + bn stats ----
            for m in range(CT):
                fs = fpool.tile([128, NPIX], F32, name="final")
                for nch in range(2):
                    fh = pbig.tile([128, 512], F32, name="fp", tag="big")
                    for k4 in range(HID // 128):
                        nc.tensor.matmul(
                            fh,
                            lhsT=wout_sb[k4][:, 128 * m:128 * (m + 1)],
                            rhs=out_sb[k4][:, 512 * nch:512 * (nch + 1)],
                            start=(k4 == 0), stop=(k4 == HID // 128 - 1))
                    fsh = fs[:, 512 * nch:512 * (nch + 1)]
                    nc.scalar.copy(fsh, fh)
                    nc.vector.bn_stats(
                        out=stats_sb[m][:, 2 * b + nch, :], in_=fsh)
                final_sb[b][m] = fs

        # ---- batch-norm: aggregate, all-reduce, normalize, store ----
        ccin = dpool.tile([128, 2 * CT], F32, name="ccin")
        ccout = dpool.tile([128, 2 * CT], F32, name="ccout")
        no_cc = os.environ.get("BASS_ATTN_NO_CC") == "1"  # timing-only builds
        # switch the ACT table to the sqrt set while PE still runs the last
        # final-proj matmuls, so the tail's Sqrt doesn't pay the ~1.3us load
        warm_sq = small.tile([1, 1], F32, name="warmsq")
        nc.scalar.activation(out=warm_sq, in_=eps_sb[0:1, :], func=AF.Sqrt)
        pk4 = small.tile([128, 2 * CT], F32, name="pk4")
        for m in range(CT):
            mv = small.tile([128, 2], F32, name="mv")
            nc.vector.bn_aggr(out=mv, in_=stats_sb[m])
            pk = pk4[:, 2 * m:2 * (m + 1)]
            nc.vector.tensor_mul(pk[:, 1:2], mv[:, 0:1], mv[:, 0:1])
            nc.vector.tensor_add(pk[:, 1:2], pk[:, 1:2], mv[:, 1:2])
            nc.vector.tensor_copy(pk[:, 0:1], mv[:, 0:1])
            nc.vector.tensor_scalar_mul(pk, in0=pk, scalar1=1.0 / N_CORES)
        nc.sync.dma_start(out=ccin, in_=pk4)
        if not no_cc:
            nc.gpsimd.collective_compute(
                "AllReduce", ALU.add,
                replica_groups=[list(range(N_CORES))],
                ins=[ccin.opt()], outs=[ccout.opt()])
        gst = small.tile([128, 2 * CT], F32, name="gst")
        nc.sync.dma_start(out=gst, in_=ccout if not no_cc else ccin)
        for m in range(CT):
            gmean = gst[:, 2 * m:2 * m + 1]
            gex2 = gst[:, 2 * m + 1:2 * m + 2]
            var = small.tile([128, 1], F32, name="var")
            nc.vector.tensor_mul(var, gmean, gmean)
            nc.vector.tensor_sub(var, gex2, var)
            std = small.tile([128, 1], F32, name="std")
            nc.scalar.activation(out=std, in_=var, func=AF.Sqrt, bias=eps_sb)
            rstd = small.tile([128, 1], F32, name="rstd")
            nc.vector.reciprocal(rstd, std)
            rsg = small.tile([128, 1], F32, name="rsg")
            nc.vector.tensor_mul(rsg, rstd, gamma_sb[m])
            sh = small.tile([128, 1], F32, name="sh")
            nc.vector.tensor_mul(sh, gmean, rsg)
            nc.vector.tensor_sub(sh, beta_sb[m], sh)
            for b in range(B):
                fs = final_sb[b][m]
                if b % 2 == 0:
                    nc.vector.tensor_scalar(
                        out=fs, in0=fs, scalar1=rsg, scalar2=sh,
                        op0=ALU.mult, op1=ALU.add)
                else:
                    nc.scalar.activation(
                        out=fs, in_=fs, func=AF.Identity,
                        bias=sh, scale=rsg)
                nc.sync.dma_start(
                    out=y[b, 128 * m:128 * (m + 1), :], in_=fs)


_CACHE = {}


def _build():
    if "nc" in _CACHE:
        return _CACHE["nc"]
    nc = bacc.Bacc("TRN2", target_bir_lowering=False, debug=False,
                   enable_asserts=True, num_devices=N_CORES)
    x = nc.dram_tensor("x", [B, C, NPIX], F32R, kind="ExternalInput").ap()
    wqkv = nc.dram_tensor("wqkvT", [C, 3 * HID], F32R,
                          kind="ExternalInput").ap()
    wout = nc.dram_tensor("woutT", [HID, C], F32R, kind="ExternalInput").ap()
    gammab = nc.dram_tensor("gammab", [C, 1], F32, kind="ExternalInput").ap()
    betab = nc.dram_tensor("betab", [C, 1], F32, kind="ExternalInput").ap()
    y = nc.dram_tensor("y", [B, C, NPIX], F32, kind="ExternalOutput").ap()
    with tile.TileContext(nc) as tc:
        _emit(tc, x, wqkv, wout, gammab, betab, y)
    nc.compile()
    _CACHE["nc"] = nc
    return nc


def kernel(x, w_qkv, w_out, b_out, gamma, beta, _trace=False):
    x = np.asarray(x, dtype=np.float32)
    wqkvT = np.ascontiguousarray(np.asarray(w_qkv, np.float32).T)   # [256, 1536]
    woutT = np.ascontiguousarray(np.asarray(w_out, np.float32).T)   # [512, 256]
    gammab = np.ascontiguousarray(np.asarray(gamma, np.float32).reshape(C, 1))
    betab = np.ascontiguousarray(np.asarray(beta, np.float32).reshape(C, 1))
    # b_out is intentionally unused: BatchNorm's mean subtraction cancels any
    # per-channel constant added before it, exactly.

    btot, c, hh, ww = x.shape
    assert (btot, c, hh * ww) == (B * N_CORES, C, NPIX)
    xf = x.reshape(btot, C, NPIX)

    nc = _build()
    in_maps = []
    for core in range(N_CORES):
        in_maps.append({
            "x": np.ascontiguousarray(xf[B * core:B * (core + 1)]),
            "wqkvT": wqkvT,
            "woutT": woutT,
            "gammab": gammab,
            "betab": betab,
        })
    res = run_bass_kernel_spmd(nc, in_maps, core_ids=list(range(N_CORES)),
                               trace=_trace)
    y = np.concatenate([res.results[core]["y"] for core in range(N_CORES)],
                       axis=0)
    out = y.reshape(btot, C, hh, ww).astype(np.float32)
    if _trace:
        kernel.last_result = res
    return out


# revision 16
# speedup vs baseline: 1.3761x; 1.3761x over previous
"""Trainium2 Bass kernel for nn_AttentionBlock (linear attention + BatchNorm).

Math (per batch, c=256 channels, n=1024 pixels, 8 heads x 64 dims):
  qkv = w_qkv @ x                      [1536, n]
  q   = softmax(q, axis=d) * d^-0.5    (per head, over the 64 head-dims)
  k   = softmax(k, axis=n)             (per head-dim, over pixels)
  ctx = k @ (v/n)^T                    [d, e] per head
  out = ctx^T @ q                      [e, n] per head
  y   = BatchNorm(w_out @ out + b_out) (batch stats over (b, n) per channel)

Sharding: data-parallel over batch across 8 cores (4 batches each); BN batch
stats are combined with a tiny AllReduce (2 floats per channel). b_out is
skipped: BatchNorm's mean subtraction cancels any per-channel constant exactly.

Device layouts (per batch):
  x      [c, n]           c on partitions (2 tiles)
  q      [(h d), n]       via lhsT=w_q^T  -> exp on ACT -> expq (f32r)
  kv^T   [n, (k|v)(h d)]  via lhsT=x      -> exp_k / v copies (fp16)
  ctx_h  [d, e+1]         contraction over n (8 chunks, PSUM accum);
                          col 64 = Zk (ones column) -> per-partition norm
  out_h  [e, n]           lhsT=ctx, rhs=expq, quadrant-packed pairs of heads
  Zq     [(h d), n]       block-mask matmul; recip folds SCALE and 1/n
  final  [c, n]           lhsT=w_out^T; bn_stats per batch; AllReduce of
                          packed (mean, E[x^2]); normalize in place; DMA out.
"""

import os
import sys

import numpy as np

for _p in ("/opt/trn_rl_repo", "/root/.axon_site/_ro/trn_rl_repo"):
    if os.path.isdir(_p) and _p not in sys.path:
        sys.path.insert(0, _p)

import concourse.bacc as bacc
import concourse.tile as tile
from concourse import mybir
from concourse.bass_utils import run_bass_kernel_spmd

F32 = mybir.dt.float32
F32R = mybir.dt.float32r
FP16 = mybir.dt.float16
AF = mybir.ActivationFunctionType
ALU = mybir.AluOpType

N_CORES = 8
# B is overridable for cheap simulator runs (BASS_ATTN_B=1 -> 8 batches total).
B = int(os.environ.get("BASS_ATTN_B", "4"))  # batches per core
C = 256          # channels
NPIX = 1024      # pixels (32*32)
H = 8            # heads
D = 64           # head dim
HID = H * D      # 512
NT = NPIX // 128  # 8 n-tiles
CT = C // 128     # 2 c-tiles
QT = HID // 128   # 4 q-tiles
SCALE = D ** -0.5
BN_EPS = 1e-5
# Zq-broadcast matmul uses this instead of 1.0 so reciprocal(Zqb) directly
# yields SCALE / (n * Zq), folding the softmax scale and the v/n factor.
MASKVAL = NPIX / SCALE


def _emit(tc, x, wqkv, wout, gammab, betab, y):
    nc = tc.nc
    from contextlib import ExitStack
    ctx_stack = ExitStack()
    with ctx_stack:
        const = ctx_stack.enter_context(tc.tile_pool(name="const", bufs=1))
        xin = ctx_stack.enter_context(tc.tile_pool(name="xin", bufs=4))
        kvsb = ctx_stack.enter_context(tc.tile_pool(name="kvsb", bufs=3))
        qpool = ctx_stack.enter_context(tc.tile_pool(name="qpool", bufs=6))
        rpool = ctx_stack.enter_context(tc.tile_pool(name="rpool", bufs=5))
        cpool = ctx_stack.enter_context(tc.tile_pool(name="cpool", bufs=8))
        opool = ctx_stack.enter_context(tc.tile_pool(name="opool", bufs=6))
        fpool = ctx_stack.enter_context(tc.tile_pool(name="fpool", bufs=2 * B))
        small = ctx_stack.enter_context(tc.tile_pool(name="small", bufs=8))
        stats_p = ctx_stack.enter_context(tc.tile_pool(name="statsp", bufs=1))
        pbig = ctx_stack.enter_context(
            tc.tile_pool(name="pbig", bufs=4, space="PSUM"))
        pctx = ctx_stack.enter_context(
            tc.tile_pool(name="pctx", bufs=4, space="PSUM"))
        dpool = ctx_stack.enter_context(
            tc.tile_pool(name="dram", bufs=1, space="DRAM"))

        # ---- constants ----
        wqkv_sb = []
        for kc in range(CT):
            w = const.tile([128, 3 * HID], F32R, name=f"wqkv{kc}")
            # kv columns first so the first batch's kv matmuls start early
            for piece in range(2):
                c0 = HID + 512 * piece
                nc.sync.dma_start(out=w[:, c0:c0 + 512],
                                  in_=wqkv[128 * kc:128 * (kc + 1), c0:c0 + 512])
            nc.sync.dma_start(out=w[:, 0:HID],
                              in_=wqkv[128 * kc:128 * (kc + 1), 0:HID])
            wqkv_sb.append(w)
        wout_sb = []
        for k4 in range(HID // 128):
            w = const.tile([128, C], F32R, name=f"wout{k4}")
            nc.sync.dma_start(out=w, in_=wout[128 * k4:128 * (k4 + 1), :])
            wout_sb.append(w)
        gamma_sb, beta_sb = [], []
        for m in range(CT):
            g = const.tile([128, 1], F32, name=f"gamma{m}")
            nc.sync.dma_start(out=g, in_=gammab[128 * m:128 * (m + 1), :])
            gamma_sb.append(g)
            bb = const.tile([128, 1], F32, name=f"beta{m}")
            nc.sync.dma_start(out=bb, in_=betab[128 * m:128 * (m + 1), :])
            beta_sb.append(bb)
        bmask = const.tile([128, 128], FP16, name="bmask")
        nc.vector.memset(bmask, 0.0)
        nc.vector.memset(bmask[0:64, 0:64], MASKVAL)
        nc.vector.memset(bmask[64:128, 64:128], MASKVAL)
        eps_sb = const.tile([128, 1], F32, name="eps")
        nc.vector.memset(eps_sb, BN_EPS)


        stats_sb = [stats_p.tile([128, 2 * B, 6], F32, name=f"stats{m}")
                    for m in range(CT)]
        final_sb = [[None] * CT for _ in range(B)]

        for b in range(B):
            xc = []
            for kc in range(CT):
                xt = xin.tile([128, NPIX], F32R, name="xc")
                for hf in range(2):
                    nc.scalar.dma_start(
                        out=xt[:, 512 * hf:512 * (hf + 1)],
                        in_=x[b, 128 * kc:128 * (kc + 1),
                              512 * hf:512 * (hf + 1)])
                xc.append(xt)

            # ---- KV projection + context accumulation over n-chunks ----
            # one PSUM bank per head-pair: only one accumulation group may be
            # open per 2KB zero region per partition
            ctxu = [pctx.tile([128, D + 1], F32, name="ctxu", tag="ctxu")
                    for _ in range(4)]
            for t in range(NT):
                halves = []
                for nch in range(2):
                    hp = pbig.tile([128, 512], F32, name="kvp", tag="big")
                    for kc in range(CT):
                        nc.tensor.matmul(
                            hp,
                            lhsT=xc[kc][:, 128 * t:128 * (t + 1)],
                            rhs=wqkv_sb[kc][:, HID + 512 * nch:
                                            HID + 512 * (nch + 1)],
                            start=(kc == 0), stop=(kc == CT - 1))
                    halves.append(hp)
                expk = kvsb.tile([128, HID], FP16, name="expk")
                nc.scalar.activation(out=expk, in_=halves[0], func=AF.Exp)
                vx = kvsb.tile([128, H, D + 1], FP16, name="vx")
                nc.vector.memset(vx[:, :, D:D + 1], 1.0)
                nc.vector.tensor_copy(
                    vx[:, :, 0:D],
                    halves[1].rearrange("p (h e) -> p h e", h=H))
                for pr in range(4):
                    for j in range(2):
                        h = 2 * pr + j
                        # skip_group_check: j=0/j=1 share the bank but write
                        # disjoint partition ranges; the sim's zero-region
                        # bookkeeping ignores partition base and would raise.
                        nc.tensor.matmul(
                            ctxu[pr][64 * j:64 * (j + 1), :],
                            lhsT=expk[:, D * h:D * (h + 1)],
                            rhs=vx[:, h, :],
                            start=(t == 0), stop=(t == NT - 1),
                            tile_position=(0, 64 * j),
                            skip_group_check=True)

            # ---- context normalization (per-partition Zk) ----
            ctx_sb = []
            for pr in range(4):
                rz = small.tile([128, 1], F32, name="rz")
                nc.vector.reciprocal_approx_fast(out=rz, in_=ctxu[pr][:, D:D + 1])
                cs = cpool.tile([128, D], FP16, name="ctxsb")
                nc.vector.tensor_scalar_mul(cs, in0=ctxu[pr][:, 0:D], scalar1=rz)
                ctx_sb.append(cs)

            # ---- Q projection, exp, Zq block-broadcast, reciprocal ----
            expq, recipb = [], []
            for t in range(QT):
                eq = qpool.tile([128, NPIX], FP16, name="expq")
                rb = rpool.tile([128, NPIX], F32, name="recipb")
                for nch in range(2):
                    qh = pbig.tile([128, 512], F32, name="qp", tag="big")
                    for kc in range(CT):
                        nc.tensor.matmul(
                            qh,
                            lhsT=wqkv_sb[kc][:, 128 * t:128 * (t + 1)],
                            rhs=xc[kc][:, 512 * nch:512 * (nch + 1)],
                            start=(kc == 0), stop=(kc == CT - 1))
                    eqh = eq[:, 512 * nch:512 * (nch + 1)]
                    nc.scalar.activation(out=eqh, in_=qh, func=AF.Exp)
                    # Zqb overwrites qh (WAR-ordered after the exp read):
                    # one 1-bank slot per half, freed right after the recip.
                    nc.tensor.matmul(qh, lhsT=bmask, rhs=eqh,
                                     start=True, stop=True)
                    nc.vector.reciprocal_approx_fast(
                        out=rb[:, 512 * nch:512 * (nch + 1)], in_=qh)
                expq.append(eq)
                recipb.append(rb)

            # ---- out = ctx^T @ expq (quadrant-packed pairs), normalize ----
            out_sb = []
            for t in range(QT):
                os_ = opool.tile([128, NPIX], F32R, name="outsb")
                for nch in range(2):
                    oh = pbig.tile([128, 512], F32, name="ou", tag="big")
                    for j in range(2):
                        # j=0/j=1 share the bank but write disjoint partition
                        # ranges (see ctxu note on skip_group_check).
                        nc.tensor.matmul(
                            oh[64 * j:64 * (j + 1), :],
                            lhsT=ctx_sb[t][64 * j:64 * (j + 1), :],
                            rhs=expq[t][64 * j:64 * (j + 1),
                                        512 * nch:512 * (nch + 1)],
                            start=True, stop=True,
                            tile_position=(64 * j, 64 * j),
                            skip_group_check=True)
                    nc.vector.tensor_mul(
                        os_[:, 512 * nch:512 * (nch + 1)], oh,
                        recipb[t][:, 512 * nch:512 * (nch + 1)])
                out_sb.append(os_)

            # ---- final projection + bn stats ----
            for m in range(CT):
                fs = fpool.tile([128, NPIX], F32, name="final")
                for nch in range(2):
                    fh = pbig.tile([128, 512], F32, name="fp", tag="big")
                    for k4 in range(HID // 128):
                        nc.tensor.matmul(
                            fh,
                            lhsT=wout_sb[k4][:, 128 * m:128 * (m + 1)],
                            rhs=out_sb[k4][:, 512 * nch:512 * (nch + 1)],
                            start=(k4 == 0), stop=(k4 == HID // 128 - 1))
                    fsh = fs[:, 512 * nch:512 * (nch + 1)]
                    nc.scalar.copy(fsh, fh)
                    nc.vector.bn_stats(
                        out=stats_sb[m][:, 2 * b + nch, :], in_=fsh)
                final_sb[b][m] = fs

        # ---- batch-norm: aggregate, all-reduce, normalize, store ----
        ccin = dpool.tile([128, 2 * CT], F32, name="ccin")
        ccout = dpool.tile([128, 2 * CT], F32, name="ccout")
        no_cc = os.environ.get("BASS_ATTN_NO_CC") == "1"  # timing-only builds
        # switch the ACT table to the sqrt set while PE still runs the last
        # final-proj matmuls, so the tail's Sqrt doesn't pay the ~1.3us load
        warm_sq = small.tile([1, 1], F32, name="warmsq")
        nc.scalar.activation(out=warm_sq, in_=eps_sb[0:1, :], func=AF.Sqrt)
        pk4 = small.tile([128, 2 * CT], F32, name="pk4")
        for m in range(CT):
            mv = small.tile([128, 2], F32, name="mv")
            nc.vector.bn_aggr(out=mv, in_=stats_sb[m])
            pk = pk4[:, 2 * m:2 * (m + 1)]
            nc.vector.tensor_mul(pk[:, 1:2], mv[:, 0:1], mv[:, 0:1])
            nc.vector.tensor_add(pk[:, 1:2], pk[:, 1:2], mv[:, 1:2])
            nc.vector.tensor_copy(pk[:, 0:1], mv[:, 0:1])
            nc.vector.tensor_scalar_mul(pk, in0=pk, scalar1=1.0 / N_CORES)
        nc.sync.dma_start(out=ccin, in_=pk4)
        if not no_cc:
            nc.gpsimd.collective_compute(
                "AllReduce", ALU.add,
                replica_groups=[list(range(N_CORES))],
                ins=[ccin.opt()], outs=[ccout.opt()])
        gst = small.tile([128, 2 * CT], F32, name="gst")
        nc.sync.dma_start(out=gst, in_=ccout if not no_cc else ccin)
        for m in range(CT):
            gmean = gst[:, 2 * m:2 * m + 1]
            gex2 = gst[:, 2 * m + 1:2 * m + 2]
            var = small.tile([128, 1], F32, name="var")
            nc.vector.tensor_mul(var, gmean, gmean)
            nc.vector.tensor_sub(var, gex2, var)
            std = small.tile([128, 1], F32, name="std")
            nc.scalar.activation(out=std, in_=var, func=AF.Sqrt, bias=eps_sb)
            rstd = small.tile([128, 1], F32, name="rstd")
            nc.vector.reciprocal_approx_fast(out=rstd, in_=std)
            rsg = small.tile([128, 1], F32, name="rsg")
            nc.vector.tensor_mul(rsg, rstd, gamma_sb[m])
            sh = small.tile([128, 1], F32, name="sh")
            nc.vector.tensor_mul(sh, gmean, rsg)
            nc.vector.tensor_sub(sh, beta_sb[m], sh)
            for b in range(B):
                fs = final_sb[b][m]
                if b % 2 == 0:
                    nc.vector.tensor_scalar(
                        out=fs, in0=fs, scalar1=rsg, scalar2=sh,
                        op0=ALU.mult, op1=ALU.add)
                else:
                    nc.scalar.activation(
                        out=fs, in_=fs, func=AF.Identity,
                        bias=sh, scale=rsg)
                nc.sync.dma_start(
                    out=y[b, 128 * m:128 * (m + 1), :], in_=fs)


_CACHE = {}


def _build():
    if "nc" in _CACHE:
        return _CACHE["nc"]
    nc = bacc.Bacc("TRN2", target_bir_lowering=False, debug=False,
                   enable_asserts=True, num_devices=N_CORES)
    x = nc.dram_tensor("x", [B, C, NPIX], F32R, kind="ExternalInput").ap()
    wqkv = nc.dram_tensor("wqkvT", [C, 3 * HID], F32R,
                          kind="ExternalInput").ap()
    wout = nc.dram_tensor("woutT", [HID, C], F32R, kind="ExternalInput").ap()
    gammab = nc.dram_tensor("gammab", [C, 1], F32, kind="ExternalInput").ap()
    betab = nc.dram_tensor("betab", [C, 1], F32, kind="ExternalInput").ap()
    y = nc.dram_tensor("y", [B, C, NPIX], F32, kind="ExternalOutput").ap()
    with tile.TileContext(nc) as tc:
        _emit(tc, x, wqkv, wout, gammab, betab, y)
    nc.compile()
    _CACHE["nc"] = nc
    return nc


def kernel(x, w_qkv, w_out, b_out, gamma, beta, _trace=False):
    x = np.asarray(x, dtype=np.float32)
    wqkvT = np.ascontiguousarray(np.asarray(w_qkv, np.float32).T)   # [256, 1536]
    woutT = np.ascontiguousarray(np.asarray(w_out, np.float32).T)   # [512, 256]
    gammab = np.ascontiguousarray(np.asarray(gamma, np.float32).reshape(C, 1))
    betab = np.ascontiguousarray(np.asarray(beta, np.float32).reshape(C, 1))
    # b_out is intentionally unused: BatchNorm's mean subtraction cancels any
    # per-channel constant added before it, exactly.

    btot, c, hh, ww = x.shape
    assert (btot, c, hh * ww) == (B * N_CORES, C, NPIX)
    xf = x.reshape(btot, C, NPIX)

    nc = _build()
    in_maps = []
    for core in range(N_CORES):
        in_maps.append({
            "x": np.ascontiguousarray(xf[B * core:B * (core + 1)]),
            "wqkvT": wqkvT,
            "woutT": woutT,
            "gammab": gammab,
            "betab": betab,
        })
    res = run_bass_kernel_spmd(nc, in_maps, core_ids=list(range(N_CORES)),
                               trace=_trace)
    y = np.concatenate([res.results[core]["y"] for core in range(N_CORES)],
                       axis=0)
    out = y.reshape(btot, C, hh, ww).astype(np.float32)
    if _trace:
        kernel.last_result = res
    return out


# revision 17
# speedup vs baseline: 1.6088x; 1.1691x over previous
"""Trainium2 Bass kernel for nn_AttentionBlock (linear attention + BatchNorm).

Math (per batch, c=256 channels, n=1024 pixels, 8 heads x 64 dims):
  qkv = w_qkv @ x                      [1536, n]
  q   = softmax(q, axis=d) * d^-0.5    (per head, over the 64 head-dims)
  k   = softmax(k, axis=n)             (per head-dim, over pixels)
  ctx = k @ (v/n)^T                    [d, e] per head
  out = ctx^T @ q                      [e, n] per head
  y   = BatchNorm(w_out @ out + b_out) (batch stats over (b, n) per channel)

Sharding: data-parallel over batch across 8 cores (4 batches each); BN batch
stats are combined with a tiny AllReduce (2 floats per channel). b_out is
skipped: BatchNorm's mean subtraction cancels any per-channel constant exactly.

Device layouts (per batch):
  x      [c, n]           c on partitions (2 tiles)
  q      [(h d), n]       via lhsT=w_q^T  -> exp on ACT -> expq (f32r)
  kv^T   [n, (k|v)(h d)]  via lhsT=x      -> exp_k / v copies (fp16)
  ctx_h  [d, e+1]         contraction over n (8 chunks, PSUM accum);
                          col 64 = Zk (ones column) -> per-partition norm
  out_h  [e, n]           lhsT=ctx, rhs=expq, quadrant-packed pairs of heads
  Zq     [(h d), n]       block-mask matmul; recip folds SCALE and 1/n
  final  [c, n]           lhsT=w_out^T; bn_stats per batch; AllReduce of
                          packed (mean, E[x^2]); normalize in place; DMA out.
"""

import os
import sys

import numpy as np

for _p in ("/opt/trn_rl_repo", "/root/.axon_site/_ro/trn_rl_repo"):
    if os.path.isdir(_p) and _p not in sys.path:
        sys.path.insert(0, _p)

import concourse.bacc as bacc
import concourse.tile as tile
from concourse import mybir
from concourse.bass_utils import run_bass_kernel_spmd

F32 = mybir.dt.float32
F32R = mybir.dt.float32r
FP16 = mybir.dt.float16
AF = mybir.ActivationFunctionType
ALU = mybir.AluOpType

N_CORES = 8
# B is overridable for cheap simulator runs (BASS_ATTN_B=1 -> 8 batches total).
B = int(os.environ.get("BASS_ATTN_B", "4"))  # batches per core
C = 256          # channels
NPIX = 1024      # pixels (32*32)
H = 8            # heads
D = 64           # head dim
HID = H * D      # 512
NT = NPIX // 128  # 8 n-tiles
CT = C // 128     # 2 c-tiles
QT = HID // 128   # 4 q-tiles
SCALE = D ** -0.5
BN_EPS = 1e-5
# Zq-broadcast matmul uses this instead of 1.0 so reciprocal(Zqb) directly
# yields SCALE / (n * Zq), folding the softmax scale and the v/n factor.
MASKVAL = NPIX / SCALE


def _emit(tc, x, wqkv, wout, gammab, betab, y):
    nc = tc.nc
    from contextlib import ExitStack
    ctx_stack = ExitStack()
    with ctx_stack:
        const = ctx_stack.enter_context(tc.tile_pool(name="const", bufs=1))
        xin = ctx_stack.enter_context(tc.tile_pool(name="xin", bufs=4))
        kvsb = ctx_stack.enter_context(tc.tile_pool(name="kvsb", bufs=3))
        qpool = ctx_stack.enter_context(tc.tile_pool(name="qpool", bufs=6))
        rpool = ctx_stack.enter_context(tc.tile_pool(name="rpool", bufs=5))
        cpool = ctx_stack.enter_context(tc.tile_pool(name="cpool", bufs=8))
        opool = ctx_stack.enter_context(tc.tile_pool(name="opool", bufs=6))
        fpool = ctx_stack.enter_context(tc.tile_pool(name="fpool", bufs=2 * B))
        small = ctx_stack.enter_context(tc.tile_pool(name="small", bufs=8))
        stats_p = ctx_stack.enter_context(tc.tile_pool(name="statsp", bufs=1))
        pbig = ctx_stack.enter_context(
            tc.tile_pool(name="pbig", bufs=4, space="PSUM"))
        pctx = ctx_stack.enter_context(
            tc.tile_pool(name="pctx", bufs=4, space="PSUM"))
        dpool = ctx_stack.enter_context(
            tc.tile_pool(name="dram", bufs=1, space="DRAM"))

        # ---- constants ----
        wqkv_sb = []
        for kc in range(CT):
            w = const.tile([128, 3 * HID], FP16, name=f"wqkv{kc}")
            # kv columns first so the first batch's kv matmuls start early
            for piece in range(2):
                c0 = HID + 512 * piece
                nc.sync.dma_start(out=w[:, c0:c0 + 512],
                                  in_=wqkv[128 * kc:128 * (kc + 1), c0:c0 + 512])
            nc.sync.dma_start(out=w[:, 0:HID],
                              in_=wqkv[128 * kc:128 * (kc + 1), 0:HID])
            wqkv_sb.append(w)
        wout_sb = []
        for k4 in range(HID // 128):
            w = const.tile([128, C], FP16, name=f"wout{k4}")
            nc.sync.dma_start(out=w, in_=wout[128 * k4:128 * (k4 + 1), :])
            wout_sb.append(w)
        gamma_sb, beta_sb = [], []
        for m in range(CT):
            g = const.tile([128, 1], F32, name=f"gamma{m}")
            nc.sync.dma_start(out=g, in_=gammab[128 * m:128 * (m + 1), :])
            gamma_sb.append(g)
            bb = const.tile([128, 1], F32, name=f"beta{m}")
            nc.sync.dma_start(out=bb, in_=betab[128 * m:128 * (m + 1), :])
            beta_sb.append(bb)
        bmask = const.tile([128, 128], FP16, name="bmask")
        nc.vector.memset(bmask, 0.0)
        nc.vector.memset(bmask[0:64, 0:64], MASKVAL)
        nc.vector.memset(bmask[64:128, 64:128], MASKVAL)
        eps_sb = const.tile([128, 1], F32, name="eps")
        nc.vector.memset(eps_sb, BN_EPS)


        # dummy collective issued up front: the first AllReduce pays a
        # ~24us ncfw rendezvous; running it early overlaps that with compute
        no_cc_warm = os.environ.get("BASS_ATTN_NO_CC") == "1"
        if not no_cc_warm:
            wrm_i = dpool.tile([128, 1], F32, name="wrm_i")
            wrm_o = dpool.tile([128, 1], F32, name="wrm_o")
            wrm_s = const.tile([128, 1], F32, name="wrm_s")
            nc.vector.memset(wrm_s, 0.0)
            nc.sync.dma_start(out=wrm_i, in_=wrm_s)
            nc.gpsimd.collective_compute(
                "AllReduce", ALU.add,
                replica_groups=[list(range(N_CORES))],
                ins=[wrm_i.opt()], outs=[wrm_o.opt()])
        stats_sb = [stats_p.tile([128, 2 * B, 6], F32, name=f"stats{m}")
                    for m in range(CT)]
        final_sb = [[None] * CT for _ in range(B)]

        for b in range(B):
            xc = []
            for kc in range(CT):
                xt = xin.tile([128, NPIX], FP16, name="xc")
                for hf in range(2):
                    nc.scalar.dma_start(
                        out=xt[:, 512 * hf:512 * (hf + 1)],
                        in_=x[b, 128 * kc:128 * (kc + 1),
                              512 * hf:512 * (hf + 1)])
                xc.append(xt)

            # ---- KV projection + context accumulation over n-chunks ----
            # one PSUM bank per head-pair: only one accumulation group may be
            # open per 2KB zero region per partition
            ctxu = [pctx.tile([128, D + 1], F32, name="ctxu", tag="ctxu")
                    for _ in range(4)]
            for t in range(NT):
                halves = []
                for nch in range(2):
                    hp = pbig.tile([128, 512], F32, name="kvp", tag="big")
                    for kc in range(CT):
                        nc.tensor.matmul(
                            hp,
                            lhsT=xc[kc][:, 128 * t:128 * (t + 1)],
                            rhs=wqkv_sb[kc][:, HID + 512 * nch:
                                            HID + 512 * (nch + 1)],
                            start=(kc == 0), stop=(kc == CT - 1))
                    halves.append(hp)
                expk = kvsb.tile([128, HID], FP16, name="expk")
                nc.scalar.activation(out=expk, in_=halves[0], func=AF.Exp)
                vx = kvsb.tile([128, H, D + 1], FP16, name="vx")
                nc.vector.memset(vx[:, :, D:D + 1], 1.0)
                nc.vector.tensor_copy(
                    vx[:, :, 0:D],
                    halves[1].rearrange("p (h e) -> p h e", h=H))
                for pr in range(4):
                    for j in range(2):
                        h = 2 * pr + j
                        # skip_group_check: j=0/j=1 share the bank but write
                        # disjoint partition ranges; the sim's zero-region
                        # bookkeeping ignores partition base and would raise.
                        nc.tensor.matmul(
                            ctxu[pr][64 * j:64 * (j + 1), :],
                            lhsT=expk[:, D * h:D * (h + 1)],
                            rhs=vx[:, h, :],
                            start=(t == 0), stop=(t == NT - 1),
                            tile_position=(0, 64 * j),
                            skip_group_check=True)

            # ---- context normalization (per-partition Zk) ----
            ctx_sb = []
            for pr in range(4):
                rz = small.tile([128, 1], F32, name="rz")
                nc.vector.reciprocal_approx_fast(out=rz, in_=ctxu[pr][:, D:D + 1])
                cs = cpool.tile([128, D], FP16, name="ctxsb")
                nc.vector.tensor_scalar_mul(cs, in0=ctxu[pr][:, 0:D], scalar1=rz)
                ctx_sb.append(cs)

            # ---- Q projection, exp, Zq block-broadcast, reciprocal ----
            expq, recipb = [], []
            for t in range(QT):
                eq = qpool.tile([128, NPIX], FP16, name="expq")
                rb = rpool.tile([128, NPIX], F32, name="recipb")
                for nch in range(2):
                    qh = pbig.tile([128, 512], F32, name="qp", tag="big")
                    for kc in range(CT):
                        nc.tensor.matmul(
                            qh,
                            lhsT=wqkv_sb[kc][:, 128 * t:128 * (t + 1)],
                            rhs=xc[kc][:, 512 * nch:512 * (nch + 1)],
                            start=(kc == 0), stop=(kc == CT - 1))
                    eqh = eq[:, 512 * nch:512 * (nch + 1)]
                    nc.scalar.activation(out=eqh, in_=qh, func=AF.Exp)
                    # Zqb overwrites qh (WAR-ordered after the exp read):
                    # one 1-bank slot per half, freed right after the recip.
                    nc.tensor.matmul(qh, lhsT=bmask, rhs=eqh,
                                     start=True, stop=True)
                    nc.vector.reciprocal_approx_fast(
                        out=rb[:, 512 * nch:512 * (nch + 1)], in_=qh)
                expq.append(eq)
                recipb.append(rb)

            # ---- out = ctx^T @ expq (quadrant-packed pairs), normalize ----
            out_sb = []
            for t in range(QT):
                os_ = opool.tile([128, NPIX], FP16, name="outsb")
                for nch in range(2):
                    oh = pbig.tile([128, 512], F32, name="ou", tag="big")
                    for j in range(2):
                        # j=0/j=1 share the bank but write disjoint partition
                        # ranges (see ctxu note on skip_group_check).
                        nc.tensor.matmul(
                            oh[64 * j:64 * (j + 1), :],
                            lhsT=ctx_sb[t][64 * j:64 * (j + 1), :],
                            rhs=expq[t][64 * j:64 * (j + 1),
                                        512 * nch:512 * (nch + 1)],
                            start=True, stop=True,
                            tile_position=(64 * j, 64 * j),
                            skip_group_check=True)
                    nc.vector.tensor_mul(
                        os_[:, 512 * nch:512 * (nch + 1)], oh,
                        recipb[t][:, 512 * nch:512 * (nch + 1)])
                out_sb.append(os_)

            # ---- final projection + bn stats ----
            for m in range(CT):
                fs = fpool.tile([128, NPIX], F32, name="final")
                for nch in range(2):
                    fh = pbig.tile([128, 512], F32, name="fp", tag="big")
                    for k4 in range(HID // 128):
                        nc.tensor.matmul(
                            fh,
                            lhsT=wout_sb[k4][:, 128 * m:128 * (m + 1)],
                            rhs=out_sb[k4][:, 512 * nch:512 * (nch + 1)],
                            start=(k4 == 0), stop=(k4 == HID // 128 - 1))
                    fsh = fs[:, 512 * nch:512 * (nch + 1)]
                    nc.scalar.copy(fsh, fh)
                    nc.vector.bn_stats(
                        out=stats_sb[m][:, 2 * b + nch, :], in_=fsh)
                final_sb[b][m] = fs

        # ---- batch-norm: aggregate, all-reduce, normalize, store ----
        ccin = dpool.tile([128, 2 * CT], F32, name="ccin")
        ccout = dpool.tile([128, 2 * CT], F32, name="ccout")
        no_cc = os.environ.get("BASS_ATTN_NO_CC") == "1"  # timing-only builds
        # switch the ACT table to the sqrt set while PE still runs the last
        # final-proj matmuls, so the tail's Sqrt doesn't pay the ~1.3us load
        warm_sq = small.tile([1, 1], F32, name="warmsq")
        nc.scalar.activation(out=warm_sq, in_=eps_sb[0:1, :], func=AF.Sqrt)
        pk4 = small.tile([128, 2 * CT], F32, name="pk4")
        for m in range(CT):
            mv = small.tile([128, 2], F32, name="mv")
            nc.vector.bn_aggr(out=mv, in_=stats_sb[m])
            pk = pk4[:, 2 * m:2 * (m + 1)]
            nc.vector.tensor_mul(pk[:, 1:2], mv[:, 0:1], mv[:, 0:1])
            nc.vector.tensor_add(pk[:, 1:2], pk[:, 1:2], mv[:, 1:2])
            nc.vector.tensor_copy(pk[:, 0:1], mv[:, 0:1])
            nc.vector.tensor_scalar_mul(pk, in0=pk, scalar1=1.0 / N_CORES)
        nc.sync.dma_start(out=ccin, in_=pk4)
        if not no_cc:
            nc.gpsimd.collective_compute(
                "AllReduce", ALU.add,
                replica_groups=[list(range(N_CORES))],
                ins=[ccin.opt()], outs=[ccout.opt()])
        gst = small.tile([128, 2 * CT], F32, name="gst")
        nc.sync.dma_start(out=gst, in_=ccout if not no_cc else ccin)
        for m in range(CT):
            gmean = gst[:, 2 * m:2 * m + 1]
            gex2 = gst[:, 2 * m + 1:2 * m + 2]
            var = small.tile([128, 1], F32, name="var")
            nc.vector.tensor_mul(var, gmean, gmean)
            nc.vector.tensor_sub(var, gex2, var)
            std = small.tile([128, 1], F32, name="std")
            nc.scalar.activation(out=std, in_=var, func=AF.Sqrt, bias=eps_sb)
            rstd = small.tile([128, 1], F32, name="rstd")
            nc.vector.reciprocal_approx_fast(out=rstd, in_=std)
            rsg = small.tile([128, 1], F32, name="rsg")
            nc.vector.tensor_mul(rsg, rstd, gamma_sb[m])
            sh = small.tile([128, 1], F32, name="sh")
            nc.vector.tensor_mul(sh, gmean, rsg)
            nc.vector.tensor_sub(sh, beta_sb[m], sh)
            for b in range(B):
                fs = final_sb[b][m]
                if b % 2 == 0:
                    nc.vector.tensor_scalar(
                        out=fs, in0=fs, scalar1=rsg, scalar2=sh,
                        op0=ALU.mult, op1=ALU.add)
                else:
                    nc.scalar.activation(
                        out=fs, in_=fs, func=AF.Identity,
                        bias=sh, scale=rsg)
                nc.sync.dma_start(
                    out=y[b, 128 * m:128 * (m + 1), :], in_=fs)


_CACHE = {}


def _build():
    if "nc" in _CACHE:
        return _CACHE["nc"]
    nc = bacc.Bacc("TRN2", target_bir_lowering=False, debug=False,
                   enable_asserts=True, num_devices=N_CORES)
    x = nc.dram_tensor("x", [B, C, NPIX], FP16, kind="ExternalInput").ap()
    wqkv = nc.dram_tensor("wqkvT", [C, 3 * HID], FP16,
                          kind="ExternalInput").ap()
    wout = nc.dram_tensor("woutT", [HID, C], FP16, kind="ExternalInput").ap()
    gammab = nc.dram_tensor("gammab", [C, 1], F32, kind="ExternalInput").ap()
    betab = nc.dram_tensor("betab", [C, 1], F32, kind="ExternalInput").ap()
    y = nc.dram_tensor("y", [B, C, NPIX], F32, kind="ExternalOutput").ap()
    with tile.TileContext(nc) as tc:
        _emit(tc, x, wqkv, wout, gammab, betab, y)
    nc.compile()
    _CACHE["nc"] = nc
    return nc


def kernel(x, w_qkv, w_out, b_out, gamma, beta, _trace=False):
    x = np.asarray(x, dtype=np.float32)
    wqkvT = np.ascontiguousarray(np.asarray(w_qkv, np.float16).T)   # [256, 1536]
    woutT = np.ascontiguousarray(np.asarray(w_out, np.float16).T)   # [512, 256]
    gammab = np.ascontiguousarray(np.asarray(gamma, np.float32).reshape(C, 1))
    betab = np.ascontiguousarray(np.asarray(beta, np.float32).reshape(C, 1))
    # b_out is intentionally unused: BatchNorm's mean subtraction cancels any
    # per-channel constant added before it, exactly.

    btot, c, hh, ww = x.shape
    assert (btot, c, hh * ww) == (B * N_CORES, C, NPIX)
    xf = x.reshape(btot, C, NPIX)

    nc = _build()
    in_maps = []
    for core in range(N_CORES):
        in_maps.append({
            "x": np.ascontiguousarray(xf[B * core:B * (core + 1)]).astype(np.float16),
            "wqkvT": wqkvT,
            "woutT": woutT,
            "gammab": gammab,
            "betab": betab,
        })
    res = run_bass_kernel_spmd(nc, in_maps, core_ids=list(range(N_CORES)),
                               trace=_trace)
    y = np.concatenate([res.results[core]["y"] for core in range(N_CORES)],
                       axis=0)
    out = y.reshape(btot, C, hh, ww).astype(np.float32)
    if _trace:
        kernel.last_result = res
    return out
